# revision 10
# baseline (speedup 1.0000x reference)
"""DeepseekVL2 MoE gate (sigmoid + grouped top-k routing) on 8 trn2 cores.

Contract: kernel(**inputs) takes the FULL unsharded inputs
  hidden_states [4, 4096, 7168] f32, weight [256, 7168] f32,
  e_score_correction_bias [256] f32
and returns (topk_idx [16384, 8] int32, topk_weight [16384, 8] f32),
matching reference jax semantics.

Strategy:
  - Data parallel: 16384 tokens -> 2048 per core x 8 cores.
  - Gating GEMM as fp16 main pass + two fp8(e4m3) DoubleRow correction
    passes (2.0 fp16-units of PE work vs 3.0 for the fp16 hi/lo 3-pass):
      logits*1024 = xh@whs + (xl*128)@(whs/128) + (xh/8)@(wl*8)
    with xh=fp16(x), xl=x-xh, whs=fp16(w.T*1024), wl=w.T*1024-whs.
    Scales put every product in the same *1024 logit scale so all three
    passes accumulate into one PSUM bank. DoubleRow packs K=256 per
    fp8 matmul at 2x PE rate. Emulated: 3/16384 token mismatches.
  - DMA co-bottleneck mitigation: the (xh/8) and (whs/128) fp8 planes
    are derived on-chip by DVE casts (pipelined a tile ahead), outputs
    are accumulated in SBUF and shipped as one DMA per output at the
    end, and the remaining 49.8MB/core is split evenly across the two
    HWDGE queues.
  - A burst of dependency-free warmup matmuls on scratch SBUF runs
    while the first DMAs land, so the PE HAM clock gate opens (1.2 ->
    2.4 GHz) before the first real matmul instead of ~25us in.
  - Routing per 128-token tile entirely on-chip with DVE max8 /
    max_index / match_replace ops (tie semantics match jax top_k).
"""

import numpy as np
import ml_dtypes

import concourse.bacc as bacc
import concourse.bass as bass
import concourse.mybir as mybir
from concourse.bass_utils import run_bass_kernel_spmd
from concourse.tile import TileContext

F16 = mybir.dt.float16
F32 = mybir.dt.float32
F8 = mybir.dt.float8e4
U32 = mybir.dt.uint32
I32 = mybir.dt.int32
E4NP = ml_dtypes.float8_e4m3

N_CORES = 8
T_FULL = 16384
T_CORE = T_FULL // N_CORES          # 2048
H = 7168
E = 256
KT = H // 128                        # 56 contraction tiles
NPAIR = KT // 2                      # 28 fp8 DoubleRow k-pairs per pass
N_TILES = T_CORE // 128              # 16 token tiles per core
KSPLIT = 41                          # xh k-tiles 0..40 on Q1, 41..55 on Q2
N_WARMUP = 12                        # HAM warmup matmuls
N_GROUP = 8
GROUP_SIZE = E // N_GROUP            # 32
TOPK_GROUP = 4
TOP_K = 8
ROUTED_SCALING = 2.5
W_SCALE = 1024.0                     # keeps wl fp16-normal
XL_S = 128.0                         # xl plane stored as e4m3(xl*128)
X8_S = 0.125                         # x8 derived on-chip as e4m3(xh/8)
NEG_BIG = -1.0e30


def _build_nc():
    nc = bacc.Bacc(
        "TRN2",
        target_bir_lowering=False,
        debug=False,
        num_devices=N_CORES,
    )

    # packed x plane, one run per (partition, tile): first 7168 f16 are
    # xh[k,t], the next 3584 f16 slots hold the 7168 xl8 fp8 bytes
    xz_d = nc.dram_tensor("xz", [128, N_TILES, 10752], F16, kind="ExternalInput").ap()
    # w16 pre-shuffled to [p, chunk, k, e]; w8 holds the wl8 DoubleRow
    # k-pairs [p, chunk, pair, ko, e] (wh8c is derived on-chip from w16).
    w16_d = nc.dram_tensor("w16", [128, 8, 7, E], F16, kind="ExternalInput").ap()
    w8_d = nc.dram_tensor("w8", [128, 4, 14, 2, E], F8, kind="ExternalInput").ap()
    bias_d = nc.dram_tensor("biasb", [128, E], F32, kind="ExternalInput").ap()
    # outputs in SBUF-native layout [p, tile, 8]; host re-folds to [T, 8]
    idx_d = nc.dram_tensor("out_idx", [128, N_TILES, TOP_K], U32, kind="ExternalOutput").ap()
    w_d = nc.dram_tensor("out_w", [128, N_TILES, TOP_K], F32, kind="ExternalOutput").ap()

    X = mybir.AxisListType.X
    Alu = mybir.AluOpType
    DR = mybir.MatmulPerfMode.DoubleRow

    with TileContext(nc) as tc:
        with (
            tc.tile_pool(name="wpool", bufs=1) as wpool,
            tc.tile_pool(name="xpool", bufs=3) as xpool,
            tc.tile_pool(name="x8pool", bufs=3) as x8pool,
            tc.tile_pool(name="spool", bufs=2) as spool,
            tc.tile_pool(name="small", bufs=2) as small,
            tc.tile_pool(name="psum", bufs=4, space="PSUM") as psum_pool,
        ):
            # HAM warmup: dependency-free matmuls on (uninitialized) scratch
            # SBUF keep the PE busy from engine boot so the clock gate opens
            # before the first real matmul. Output PSUM is never read.
            wsc = wpool.tile([128, E], F16, tag="wsc")
            nc.gpsimd.memset(wsc[:], 0.0)
            psw = psum_pool.tile([128, E], F32, tag="psw", name="psw")
            for i in range(N_WARMUP):
                nc.tensor.matmul(
                    psw[:], wsc[:, 0:128], wsc[:],
                    start=True, stop=True, skip_group_check=True,
                )

            bias_sb = wpool.tile([128, E], F32, tag="bias")
            w16c = [
                wpool.tile([128, 7, E], F16, tag=f"w16c{c}", name=f"w16c{c}")
                for c in range(8)
            ]
            w8c = [
                wpool.tile([128, 14, 2, E], F8, tag=f"w8c{c}", name=f"w8c{c}")
                for c in range(4)
            ]
            # output accumulators, one slot per tile
            idxacc = wpool.tile([128, N_TILES, TOP_K], U32, tag="idxacc")
            wacc = wpool.tile([128, N_TILES, TOP_K], F32, tag="wacc")

            def w16_k(k):
                return w16c[k // 7][:, k % 7, :]

            def load_tile(tt, xz_t):
                if tt == 0:
                    # Ramp: interleave tile-0 pieces with weight chunks in
                    # first-use order across both queues.
                    nc.sync.dma_start(xz_t[:, 0:1792], xz_d[:, 0, 0:1792])
                    nc.sync.dma_start(w16c[0][:], w16_d[:, 0, :, :])
                    nc.sync.dma_start(w16c[1][:], w16_d[:, 1, :, :])
                    nc.sync.dma_start(xz_t[:, 1792:3584], xz_d[:, 0, 1792:3584])
                    for c in range(2, 8):
                        nc.sync.dma_start(w16c[c][:], w16_d[:, c, :, :])
                    nc.scalar.dma_start(xz_t[:, 3584:7168], xz_d[:, 0, 3584:7168])
                    nc.scalar.dma_start(xz_t[:, 7168:10752], xz_d[:, 0, 7168:10752])
                    nc.scalar.dma_start(w8c[0][:], w8_d[:, 0, :, :, :])
                    nc.scalar.dma_start(w8c[1][:], w8_d[:, 1, :, :, :])
                    nc.scalar.dma_start(w8c[2][:], w8_d[:, 2, :, :, :])
                    nc.scalar.dma_start(w8c[3][:], w8_d[:, 3, :, :, :])
                    nc.scalar.dma_start(bias_sb[:], bias_d)
                elif tt <= 2:
                    # halves across both queues while the ramp backlog drains
                    nc.sync.dma_start(xz_t[:, 0:5376], xz_d[:, tt, 0:5376])
                    nc.scalar.dma_start(xz_t[:, 5376:10752], xz_d[:, tt, 5376:10752])
                else:
                    # whole-tile single DMA (21.5KB per-partition run),
                    # alternating queues for byte balance
                    q = nc.sync if tt % 2 == 1 else nc.scalar
                    q.dma_start(xz_t[:], xz_d[:, tt, :])

            def xviews(xz_t):
                xh_v = xz_t[:, 0:7168].rearrange("p (k t) -> p k t", k=KT)
                xl8_v = (
                    xz_t[:, 7168:10752].bitcast(F8)
                    .rearrange("p (k t) -> p k t", k=KT)
                )
                return xh_v, xl8_v

            # prologue: tile 0 loads + tile-0 x8 derivation
            xz_tiles = {}
            x8_tiles = {}
            xz_tiles[0] = xpool.tile([128, 10752], F16, tag="xz", name="xz0")
            load_tile(0, xz_tiles[0])
            x8_tiles[0] = x8pool.tile([128, KT, 128], F8, tag="x8", name="x80")
            nc.vector.tensor_scalar(
                x8_tiles[0][:], xviews(xz_tiles[0])[0], X8_S, None, op0=Alu.mult
            )

            for tt in range(N_TILES):
                xh_t, xl8_t = xviews(xz_tiles[tt])
                x8_t = x8_tiles[tt]
                # prefetch DMAs for the next tile (engine queues run ahead)
                if tt + 1 < N_TILES:
                    xz_tiles[tt + 1] = xpool.tile(
                        [128, 10752], F16, tag="xz", name=f"xz{tt+1}"
                    )
                    load_tile(tt + 1, xz_tiles[tt + 1])

                # PSUM accumulator [128, 256]: fp16 main pass + two fp8
                # DoubleRow correction passes, all in the same logit scale.
                ps = psum_pool.tile([128, E], F32, tag="ps")
                for k in range(KT):
                    nc.tensor.matmul(
                        ps[:], xh_t[:, k, :], w16_k(k),
                        start=(k == 0), stop=False, skip_group_check=True,
                    )
                for j in range(NPAIR):  # xl8 @ wh8c
                    nc.tensor.matmul(
                        ps[:], xl8_t[:, 2 * j : 2 * j + 2, :],
                        w8c[j // 14][:, j % 14, :, :],
                        start=False, stop=False, perf_mode=DR,
                        skip_group_check=True,
                    )
                for j in range(NPAIR):  # x8 @ wl8
                    nc.tensor.matmul(
                        ps[:], x8_t[:, 2 * j : 2 * j + 2, :],
                        w8c[2 + j // 14][:, j % 14, :, :],
                        start=False, stop=(j == NPAIR - 1), perf_mode=DR,
                        skip_group_check=True,
                    )

                # scores = sigmoid(logits) with the 1/1024 scale folded in
                scores = spool.tile([128, E], F32, tag="scores")
                nc.scalar.activation(
                    scores[:], ps[:],
                    mybir.ActivationFunctionType.Sigmoid,
                    scale=1.0 / W_SCALE,
                )

                # next tile's x8 derivation goes on the DVE queue BEFORE this
                # tile's routing chain, so it completes well before the next
                # tile's fp8 w-correction matmuls need it.
                if tt + 1 < N_TILES:
                    x8_tiles[tt + 1] = x8pool.tile(
                        [128, KT, 128], F8, tag="x8", name=f"x8{tt+1}"
                    )
                    nc.vector.tensor_scalar(
                        x8_tiles[tt + 1][:], xviews(xz_tiles[tt + 1])[0], X8_S,
                        None, op0=Alu.mult,
                    )

                # scores_for_choice = scores + bias  (bias varies along free dim)
                sfc = spool.tile([128, E], F32, tag="sfc")
                nc.vector.tensor_add(sfc[:], scores[:], bias_sb[:])

                # per-group top-2 sum: g1 = grouped max, remove it, g2 = grouped max
                sfc_g = sfc[:].rearrange("p (g e) -> p g e", g=N_GROUP)
                g1 = small.tile([128, N_GROUP], F32, tag="g1")
                nc.vector.reduce_max(g1[:], sfc_g, axis=X)
                sfc_mr = spool.tile([128, E], F32, tag="scratch", name="sfc_mr")
                nc.vector.match_replace(sfc_mr[:], g1[:], sfc[:], NEG_BIG)
                g2 = small.tile([128, N_GROUP], F32, tag="g2")
                nc.vector.reduce_max(
                    g2[:], sfc_mr[:].rearrange("p (g e) -> p g e", g=N_GROUP), axis=X
                )
                gs = small.tile([128, N_GROUP], F32, tag="gs")
                nc.vector.tensor_add(gs[:], g1[:], g2[:])

                # top-4 groups: tau = 4th largest group score -> 0/1 mask
                gsrt = small.tile([128, 8], F32, tag="gsrt")
                nc.vector.max(out=gsrt[:], in_=gs[:])
                gmask = small.tile([128, N_GROUP], F32, tag="gmask")
                nc.vector.tensor_scalar(
                    gmask[:], gs[:], gsrt[:, TOPK_GROUP - 1 : TOPK_GROUP], None,
                    op0=Alu.is_ge,
                )

                # tmp = sfc * mask (expanded over the 32 experts of each group)
                tmp = spool.tile([128, E], F32, tag="tmp")
                nc.vector.tensor_mul(
                    tmp[:].rearrange("p (g e) -> p g e", g=N_GROUP),
                    sfc_g,
                    gmask[:].unsqueeze(2).to_broadcast([128, N_GROUP, GROUP_SIZE]),
                )

                # ordered top-8 of tmp (+ indices, jax tie order); indices land
                # directly in the output accumulator slot for this tile
                v8 = small.tile([128, 8], F32, tag="v8")
                nc.vector.max(out=v8[:], in_=tmp[:])
                nc.vector.max_index(idxacc[:, tt, :], v8[:], tmp[:])

                # mark the selected positions, pull raw sigmoid scores there
                tmp_mr = spool.tile([128, E], F32, tag="scratch", name="tmp_mr")
                nc.vector.match_replace(tmp_mr[:], v8[:], tmp[:], NEG_BIG)
                sel = spool.tile([128, E], F32, tag="sel")
                nc.vector.tensor_scalar(
                    sel[:], tmp_mr[:], NEG_BIG, None, op0=Alu.is_equal
                )
                scsel = spool.tile([128, E], F32, tag="scsel")
                nc.vector.tensor_mul(scsel[:], scores[:], sel[:])
                s8 = small.tile([128, 8], F32, tag="s8")
                nc.vector.max(out=s8[:], in_=scsel[:])
                s8i = small.tile([128, 8], U32, tag="s8i")
                nc.vector.max_index(s8i[:], s8[:], scsel[:])

                # re-pair score values to sfc order: w8[k] = sum_j s8[j]*(s8i[j]==i8[k])
                e8 = small.tile([128, 8, 8], F32, tag="e8")
                nc.vector.tensor_tensor(
                    e8[:],
                    s8i[:].unsqueeze(1).to_broadcast([128, 8, 8]),
                    idxacc[:, tt, :].unsqueeze(2).to_broadcast([128, 8, 8]),
                    op=Alu.is_equal,
                )
                w64 = small.tile([128, 8, 8], F32, tag="w64")
                nc.vector.tensor_mul(
                    w64[:], e8[:], s8[:].unsqueeze(1).to_broadcast([128, 8, 8])
                )
                w8v = small.tile([128, 8], F32, tag="w8v")
                nc.vector.reduce_sum(w8v[:], w64[:], axis=X)

                # normalize: w = w8v / sum * 2.5 (the reference's +1e-20 is
                # below fp32 ulp of the denominator, which is always >1)
                ds = small.tile([128, 1], F32, tag="ds")
                nc.vector.reduce_sum(ds[:], s8[:], axis=X)
                rcp = small.tile([128, 1], F32, tag="rcp")
                nc.vector.reciprocal(rcp[:], ds[:])
                nc.vector.tensor_scalar(
                    wacc[:, tt, :], w8v[:], rcp[:, 0:1], ROUTED_SCALING,
                    op0=Alu.mult, op1=Alu.mult,
                )

            # single output DMA per tensor (keeps the HWDGE queues clean)
            nc.scalar.dma_start(idx_d[:], idxacc[:])
            nc.scalar.dma_start(w_d[:], wacc[:])

    nc.compile()
    return nc


_NC_CACHE = None


def _get_nc():
    global _NC_CACHE
    if _NC_CACHE is None:
        _NC_CACHE = _build_nc()
    return _NC_CACHE


def _prep_inputs(hidden_states, weight, e_score_correction_bias):
    x = np.ascontiguousarray(hidden_states, dtype=np.float32).reshape(T_FULL, H)
    wT = np.ascontiguousarray(weight, dtype=np.float32).T * W_SCALE  # [H, E]
    whs = wT.astype(np.float16)
    wl = wT - whs.astype(np.float32)
    w16_dev = np.ascontiguousarray(
        whs.reshape(8, 7, 128, E).transpose(2, 0, 1, 3)
    )
    # w8: DoubleRow pairs [p, chunk, pair, ko, e]; 28 wh8c pairs + 28 wl8
    wh8c = (whs.astype(np.float32) / XL_S).astype(E4NP)
    wl8 = (wl / X8_S).astype(E4NP)
    w8_all = np.concatenate(
        [wh8c.reshape(NPAIR, 2, 128, E), wl8.reshape(NPAIR, 2, 128, E)], axis=0
    )
    w8_dev = np.ascontiguousarray(
        w8_all.reshape(4, 14, 2, 128, E).transpose(3, 0, 1, 2, 4)
    )
    bias_b = np.ascontiguousarray(
        np.broadcast_to(
            np.asarray(e_score_correction_bias, dtype=np.float32)[None, :], (128, E)
        )
    )
    in_maps = []
    for c in range(N_CORES):
        xc = x[c * T_CORE : (c + 1) * T_CORE]  # [Tc, H] contiguous
        xh = xc.astype(np.float16)
        xl = xc - xh.astype(np.float32)
        xl8 = (xl * XL_S).astype(E4NP)
        # device layout [p, tile, k, t]: x[tt*128+t, k*128+p] -> A[p, tt, k, t]
        xh_dev = np.ascontiguousarray(
            xh.reshape(N_TILES, 128, KT, 128).transpose(3, 0, 2, 1)
        )
        xl8_dev = np.ascontiguousarray(
            xl8.reshape(N_TILES, 128, KT, 128).transpose(3, 0, 2, 1)
        )
        # pack per (p, tile): 14336B of xh then 7168B of xl8
        xz_dev = np.concatenate(
            [
                xh_dev.view(np.uint8).reshape(128, N_TILES, 14336),
                xl8_dev.view(np.uint8).reshape(128, N_TILES, 7168),
            ],
            axis=2,
        ).view(np.float16)
        in_maps.append(
            {
                "xz": xz_dev,
                "w16": w16_dev,
                "w8": w8_dev,
                "biasb": bias_b,
            }
        )
    return in_maps


def run(hidden_states, weight, e_score_correction_bias, trace=False, **spmd_kwargs):
    nc = _get_nc()
    in_maps = _prep_inputs(hidden_states, weight, e_score_correction_bias)
    res = run_bass_kernel_spmd(
        nc, in_maps, core_ids=list(range(N_CORES)), trace=trace, **spmd_kwargs
    )
    # outputs are [p, tile, 8] per core; token t = tile*128 + p
    idx = np.concatenate(
        [r["out_idx"].transpose(1, 0, 2).reshape(T_CORE, TOP_K) for r in res.results],
        axis=0,
    )
    w = np.concatenate(
        [r["out_w"].transpose(1, 0, 2).reshape(T_CORE, TOP_K) for r in res.results],
        axis=0,
    )
    return (idx.astype(np.int32), w.astype(np.float32)), res


def kernel(hidden_states, weight, e_score_correction_bias):
    (idx, w), _ = run(hidden_states, weight, e_score_correction_bias, trace=False)
    return idx, w


# revision 12
# speedup vs baseline: 1.0098x; 1.0098x over previous
"""DeepseekVL2 MoE gate (sigmoid + grouped top-k routing) on 8 trn2 cores.

Contract: kernel(**inputs) takes the FULL unsharded inputs
  hidden_states [4, 4096, 7168] f32, weight [256, 7168] f32,
  e_score_correction_bias [256] f32
and returns (topk_idx [16384, 8] int32, topk_weight [16384, 8] f32),
matching reference jax semantics.

Strategy:
  - Data parallel: 16384 tokens -> 2048 per core x 8 cores.
  - Gating GEMM as fp16 main pass + two fp8(e4m3) DoubleRow correction
    passes (2.0 fp16-units of PE work vs 3.0 for the fp16 hi/lo 3-pass):
      logits*1024 = xh@whs + (xl*128)@(whs/128) + (xh/8)@(wl*8)
    with xh=fp16(x), xl=x-xh, whs=fp16(w.T*1024), wl=w.T*1024-whs.
    Scales put every product in the same *1024 logit scale so all three
    passes accumulate into one PSUM bank. DoubleRow packs K=256 per
    fp8 matmul at 2x PE rate. Emulated: 3/16384 token mismatches.
    The (xh/8) fp8 operand is derived on-chip by a DVE cast pipelined
    ahead of its consumer; the (whs/128) plane ships from host (an
    on-chip cast of it was measurably less accurate).
  - DMA and tensor are nearly balanced (~50MB/core vs ~197us of PE
    work), so the schedule is built around the ramp: a chained warmup
    matmul burst opens the HAM clock gate (1.2 -> 2.4 GHz) before the
    first real matmul; tiles 0-2 run their fp16 pass first while only
    w16 + the fp16 half of their data has landed, with their fp8
    correction passes deferred (PSUM groups stay open) until the fp8
    weight planes arrive; x tiles ship as one packed plane (xh bytes
    then xl8 bytes -> single 21.5KB/partition run) split across the
    two HWDGE queues in measured-rate proportion.
  - Outputs accumulate in SBUF and leave as one DMA per tensor.
  - Routing per 128-token tile entirely on-chip with DVE max8 /
    max_index / match_replace ops (tie semantics match jax top_k).
"""

import numpy as np
import ml_dtypes

import concourse.bacc as bacc
import concourse.bass as bass
import concourse.mybir as mybir
from concourse.bass_utils import run_bass_kernel_spmd
from concourse.tile import TileContext

F16 = mybir.dt.float16
F32 = mybir.dt.float32
F8 = mybir.dt.float8e4
U32 = mybir.dt.uint32
I32 = mybir.dt.int32
E4NP = ml_dtypes.float8_e4m3

N_CORES = 8
T_FULL = 16384
T_CORE = T_FULL // N_CORES          # 2048
H = 7168
E = 256
KT = H // 128                        # 56 contraction tiles
NPAIR = KT // 2                      # 28 fp8 DoubleRow k-pairs per pass
N_TILES = T_CORE // 128              # 16 token tiles per core
N_WARMUP = 12                        # chained HAM warmup matmuls
N_GROUP = 8
GROUP_SIZE = E // N_GROUP            # 32
TOPK_GROUP = 4
TOP_K = 8
ROUTED_SCALING = 2.5
W_SCALE = 1024.0                     # keeps wl fp16-normal
XL_S = 128.0                         # xl plane stored as e4m3(xl*128)
X8_S = 0.125                         # x8 derived on-chip as e4m3(xh/8)
NEG_BIG = -1.0e30

# tensor-pass emission order: tiles 0-2 fp16-first (their fp8 passes are
# deferred until the fp8 weight planes have landed), stagger closed by t=8
_SCHED = (
    [("f16", 0), ("f16", 1), ("f16", 2), ("f8", 0), ("f16", 3), ("f8", 1),
     ("f16", 4), ("f8", 2), ("f16", 5), ("f8", 3), ("f8", 4), ("f16", 6),
     ("f8", 5), ("f16", 7), ("f8", 6), ("f16", 8), ("f8", 7), ("f8", 8)]
    + [p for t in range(9, N_TILES) for p in (("f16", t), ("f8", t))]
)


def _build_nc():
    nc = bacc.Bacc(
        "TRN2",
        target_bir_lowering=False,
        debug=False,
        num_devices=N_CORES,
    )

    # packed x plane, one run per (partition, tile): first 7168 f16 are
    # xh[k,t], the next 3584 f16 slots hold the 7168 xl8 fp8 bytes
    xz_d = nc.dram_tensor("xz", [128, N_TILES, 10752], F16, kind="ExternalInput").ap()
    # w16 pre-shuffled to [p, chunk, k, e]; w8 holds DoubleRow k-pairs
    # [p, chunk, pair, ko, e] with pairs 0..27 = wh8c, 28..55 = wl8.
    w16_d = nc.dram_tensor("w16", [128, 8, 7, E], F16, kind="ExternalInput").ap()
    w8_d = nc.dram_tensor("w8", [128, 4, 14, 2, E], F8, kind="ExternalInput").ap()
    bias_d = nc.dram_tensor("biasb", [128, E], F32, kind="ExternalInput").ap()
    # outputs in SBUF-native layout [p, tile, 8]; host re-folds to [T, 8]
    idx_d = nc.dram_tensor("out_idx", [128, N_TILES, TOP_K], U32, kind="ExternalOutput").ap()
    w_d = nc.dram_tensor("out_w", [128, N_TILES, TOP_K], F32, kind="ExternalOutput").ap()

    X = mybir.AxisListType.X
    Alu = mybir.AluOpType
    DR = mybir.MatmulPerfMode.DoubleRow

    with TileContext(nc) as tc:
        with (
            tc.tile_pool(name="wpool", bufs=1) as wpool,
            tc.tile_pool(name="xpool", bufs=3) as xpool,
            tc.tile_pool(name="x8pool", bufs=3) as x8pool,
            tc.tile_pool(name="spool", bufs=2) as spool,
            tc.tile_pool(name="small", bufs=2) as small,
            tc.tile_pool(name="psum", bufs=4, space="PSUM") as psum_pool,
        ):
            # HAM warmup: one chained accumulation of dependency-free matmuls
            # on zeroed scratch SBUF gives a contiguous ~5us busy window from
            # engine boot, so the clock gate opens before the first real
            # matmul. The PSUM result is never read.
            wsc = wpool.tile([128, 2 * E], F16, tag="wsc")
            nc.gpsimd.memset(wsc[:], 0.0)
            psw = psum_pool.tile([128, 2 * E], F32, tag="psw", name="psw")
            for i in range(N_WARMUP):
                nc.tensor.matmul(
                    psw[:], wsc[:, 0:128], wsc[:],
                    start=(i == 0), stop=(i == N_WARMUP - 1),
                    skip_group_check=True,
                )

            bias_sb = wpool.tile([128, E], F32, tag="bias")
            w16c = [
                wpool.tile([128, 7, E], F16, tag=f"w16c{c}", name=f"w16c{c}")
                for c in range(8)
            ]
            w8c = [
                wpool.tile([128, 14, 2, E], F8, tag=f"w8c{c}", name=f"w8c{c}")
                for c in range(4)
            ]
            # output accumulators, one slot per tile
            idxacc = wpool.tile([128, N_TILES, TOP_K], U32, tag="idxacc")
            wacc = wpool.tile([128, N_TILES, TOP_K], F32, tag="wacc")

            def w16_k(k):
                return w16c[k // 7][:, k % 7, :]

            def load_tile(tt, xz_t):
                if tt == 0:
                    # Ramp: interleave tile-0 pieces with weight chunks in
                    # first-use order across both queues.
                    nc.sync.dma_start(xz_t[:, 0:1792], xz_d[:, 0, 0:1792])
                    nc.sync.dma_start(w16c[0][:], w16_d[:, 0, :, :])
                    nc.sync.dma_start(w16c[1][:], w16_d[:, 1, :, :])
                    nc.sync.dma_start(xz_t[:, 1792:3584], xz_d[:, 0, 1792:3584])
                    for c in range(2, 8):
                        nc.sync.dma_start(w16c[c][:], w16_d[:, c, :, :])
                    nc.scalar.dma_start(xz_t[:, 3584:7168], xz_d[:, 0, 3584:7168])
                    nc.scalar.dma_start(xz_t[:, 7168:10752], xz_d[:, 0, 7168:10752])
                    nc.scalar.dma_start(w8c[0][:], w8_d[:, 0, :, :, :])
                    nc.scalar.dma_start(w8c[1][:], w8_d[:, 1, :, :, :])
                    nc.scalar.dma_start(w8c[2][:], w8_d[:, 2, :, :, :])
                    nc.scalar.dma_start(w8c[3][:], w8_d[:, 3, :, :, :])
                    nc.scalar.dma_start(bias_sb[:], bias_d)
                elif tt <= 2:
                    # split at the f16/f8 boundary: the fp16 pass can start on
                    # Q1's piece alone while the ramp backlog drains
                    nc.sync.dma_start(xz_t[:, 0:7168], xz_d[:, tt, 0:7168])
                    nc.scalar.dma_start(xz_t[:, 7168:10752], xz_d[:, tt, 7168:10752])
                else:
                    # whole-tile single DMA (21.5KB per-partition run); Q1 is
                    # measurably ~25% faster than Q10, so it takes the odd
                    # tiles (7 of 13) plus the heavier ramp share
                    q = nc.sync if tt % 2 == 1 else nc.scalar
                    q.dma_start(xz_t[:], xz_d[:, tt, :])

            def xviews(xz_t):
                xh_v = xz_t[:, 0:7168].rearrange("p (k t) -> p k t", k=KT)
                xl8_v = (
                    xz_t[:, 7168:10752].bitcast(F8)
                    .rearrange("p (k t) -> p k t", k=KT)
                )
                return xh_v, xl8_v

            # prologue: loads + eager x8 casts for tiles 0..2
            xz_tiles = {}
            x8_tiles = {}
            ps_tiles = {}
            for t in range(3):
                xz_tiles[t] = xpool.tile([128, 10752], F16, tag="xz", name=f"xz{t}")
                load_tile(t, xz_tiles[t])
                x8_tiles[t] = x8pool.tile([128, KT, 128], F8, tag="x8", name=f"x8{t}")
                nc.vector.tensor_scalar(
                    x8_tiles[t][:], xviews(xz_tiles[t])[0], X8_S, None, op0=Alu.mult
                )

            def emit_f16(tt):
                if tt + 1 < N_TILES and tt + 1 not in xz_tiles:
                    xz_tiles[tt + 1] = xpool.tile(
                        [128, 10752], F16, tag="xz", name=f"xz{tt+1}"
                    )
                    load_tile(tt + 1, xz_tiles[tt + 1])
                xh_v, _ = xviews(xz_tiles[tt])
                ps = psum_pool.tile([128, E], F32, tag="ps", name=f"ps{tt}")
                ps_tiles[tt] = ps
                for k in range(KT):
                    nc.tensor.matmul(
                        ps[:], xh_v[:, k, :], w16_k(k),
                        start=(k == 0), stop=False, skip_group_check=True,
                    )

            def emit_f8_and_route(tt):
                _, xl8_v = xviews(xz_tiles[tt])
                x8_t = x8_tiles[tt]
                ps = ps_tiles[tt]
                for j in range(NPAIR):  # xl8 @ wh8c
                    nc.tensor.matmul(
                        ps[:], xl8_v[:, 2 * j : 2 * j + 2, :],
                        w8c[j // 14][:, j % 14, :, :],
                        start=False, stop=False, perf_mode=DR,
                        skip_group_check=True,
                    )
                for j in range(NPAIR):  # x8 @ wl8
                    nc.tensor.matmul(
                        ps[:], x8_t[:, 2 * j : 2 * j + 2, :],
                        w8c[2 + j // 14][:, j % 14, :, :],
                        start=False, stop=(j == NPAIR - 1), perf_mode=DR,
                        skip_group_check=True,
                    )

                # scores = sigmoid(logits) with the 1/1024 scale folded in
                scores = spool.tile([128, E], F32, tag="scores")
                nc.scalar.activation(
                    scores[:], ps[:],
                    mybir.ActivationFunctionType.Sigmoid,
                    scale=1.0 / W_SCALE,
                )

                # x8 cast for tile tt+2 goes on the DVE queue BEFORE this
                # tile's routing chain, so it is ready well before that
                # tile's fp8 w-correction matmuls (which can directly follow
                # this tile's in the closing-stagger schedule).
                tn = tt + 2
                if tn < N_TILES and tn not in xz_tiles:
                    xz_tiles[tn] = xpool.tile(
                        [128, 10752], F16, tag="xz", name=f"xz{tn}"
                    )
                    load_tile(tn, xz_tiles[tn])
                if tn < N_TILES and tn not in x8_tiles:
                    x8_tiles[tn] = x8pool.tile(
                        [128, KT, 128], F8, tag="x8", name=f"x8{tn}"
                    )
                    nc.vector.tensor_scalar(
                        x8_tiles[tn][:], xviews(xz_tiles[tn])[0], X8_S, None,
                        op0=Alu.mult,
                    )

                # scores_for_choice = scores + bias (bias varies along free dim)
                sfc = spool.tile([128, E], F32, tag="sfc")
                nc.vector.tensor_add(sfc[:], scores[:], bias_sb[:])

                # per-group top-2 sum: g1 = grouped max, remove it, g2 = max
                sfc_g = sfc[:].rearrange("p (g e) -> p g e", g=N_GROUP)
                g1 = small.tile([128, N_GROUP], F32, tag="g1")
                nc.vector.reduce_max(g1[:], sfc_g, axis=X)
                sfc_mr = spool.tile([128, E], F32, tag="scratch", name="sfc_mr")
                nc.vector.match_replace(sfc_mr[:], g1[:], sfc[:], NEG_BIG)
                g2 = small.tile([128, N_GROUP], F32, tag="g2")
                nc.vector.reduce_max(
                    g2[:], sfc_mr[:].rearrange("p (g e) -> p g e", g=N_GROUP), axis=X
                )
                gs = small.tile([128, N_GROUP], F32, tag="gs")
                nc.vector.tensor_add(gs[:], g1[:], g2[:])

                # top-4 groups: tau = 4th largest group score -> 0/1 mask
                gsrt = small.tile([128, 8], F32, tag="gsrt")
                nc.vector.max(out=gsrt[:], in_=gs[:])
                gmask = small.tile([128, N_GROUP], F32, tag="gmask")
                nc.vector.tensor_scalar(
                    gmask[:], gs[:], gsrt[:, TOPK_GROUP - 1 : TOPK_GROUP], None,
                    op0=Alu.is_ge,
                )

                # tmp = sfc * mask (expanded over the 32 experts of each group)
                tmp = spool.tile([128, E], F32, tag="tmp")
                nc.vector.tensor_mul(
                    tmp[:].rearrange("p (g e) -> p g e", g=N_GROUP),
                    sfc_g,
                    gmask[:].unsqueeze(2).to_broadcast([128, N_GROUP, GROUP_SIZE]),
                )

                # ordered top-8 of tmp (+ indices, jax tie order); indices land
                # directly in the output accumulator slot for this tile
                v8 = small.tile([128, 8], F32, tag="v8")
                nc.vector.max(out=v8[:], in_=tmp[:])
                nc.vector.max_index(idxacc[:, tt, :], v8[:], tmp[:])

                # mark the selected positions, pull raw sigmoid scores there
                tmp_mr = spool.tile([128, E], F32, tag="scratch", name="tmp_mr")
                nc.vector.match_replace(tmp_mr[:], v8[:], tmp[:], NEG_BIG)
                sel = spool.tile([128, E], F32, tag="sel")
                nc.vector.tensor_scalar(
                    sel[:], tmp_mr[:], NEG_BIG, None, op0=Alu.is_equal
                )
                scsel = spool.tile([128, E], F32, tag="scsel")
                nc.vector.tensor_mul(scsel[:], scores[:], sel[:])
                s8 = small.tile([128, 8], F32, tag="s8")
                nc.vector.max(out=s8[:], in_=scsel[:])
                s8i = small.tile([128, 8], U32, tag="s8i")
                nc.vector.max_index(s8i[:], s8[:], scsel[:])

                # re-pair scores to sfc order: w8[k] = sum_j s8[j]*(s8i[j]==i8[k])
                e8 = small.tile([128, 8, 8], F32, tag="e8")
                nc.vector.tensor_tensor(
                    e8[:],
                    s8i[:].unsqueeze(1).to_broadcast([128, 8, 8]),
                    idxacc[:, tt, :].unsqueeze(2).to_broadcast([128, 8, 8]),
                    op=Alu.is_equal,
                )
                w64 = small.tile([128, 8, 8], F32, tag="w64")
                nc.vector.tensor_mul(
                    w64[:], e8[:], s8[:].unsqueeze(1).to_broadcast([128, 8, 8])
                )
                w8v = small.tile([128, 8], F32, tag="w8v")
                nc.vector.reduce_sum(w8v[:], w64[:], axis=X)

                # normalize: w = w8v / sum * 2.5 (the reference's +1e-20 is
                # below fp32 ulp of the denominator, which is always >1)
                ds = small.tile([128, 1], F32, tag="ds")
                nc.vector.reduce_sum(ds[:], s8[:], axis=X)
                rcp = small.tile([128, 1], F32, tag="rcp")
                nc.vector.reciprocal(rcp[:], ds[:])
                nc.vector.tensor_scalar(
                    wacc[:, tt, :], w8v[:], rcp[:, 0:1], ROUTED_SCALING,
                    op0=Alu.mult, op1=Alu.mult,
                )

            for kind, tt in _SCHED:
                if kind == "f16":
                    emit_f16(tt)
                else:
                    emit_f8_and_route(tt)

            # single output DMA per tensor (keeps the HWDGE queues clean)
            nc.scalar.dma_start(idx_d[:], idxacc[:])
            nc.scalar.dma_start(w_d[:], wacc[:])

    nc.compile()
    return nc


_NC_CACHE = None


def _get_nc():
    global _NC_CACHE
    if _NC_CACHE is None:
        _NC_CACHE = _build_nc()
    return _NC_CACHE


def _prep_inputs(hidden_states, weight, e_score_correction_bias):
    x = np.ascontiguousarray(hidden_states, dtype=np.float32).reshape(T_FULL, H)
    wT = np.ascontiguousarray(weight, dtype=np.float32).T * W_SCALE  # [H, E]
    whs = wT.astype(np.float16)
    wl = wT - whs.astype(np.float32)
    w16_dev = np.ascontiguousarray(
        whs.reshape(8, 7, 128, E).transpose(2, 0, 1, 3)
    )
    # w8: DoubleRow pairs [p, chunk, pair, ko, e]; 28 wh8c pairs + 28 wl8
    wh8c = (whs.astype(np.float32) / XL_S).astype(E4NP)
    wl8 = (wl / X8_S).astype(E4NP)
    w8_all = np.concatenate(
        [wh8c.reshape(NPAIR, 2, 128, E), wl8.reshape(NPAIR, 2, 128, E)], axis=0
    )
    w8_dev = np.ascontiguousarray(
        w8_all.reshape(4, 14, 2, 128, E).transpose(3, 0, 1, 2, 4)
    )
    bias_b = np.ascontiguousarray(
        np.broadcast_to(
            np.asarray(e_score_correction_bias, dtype=np.float32)[None, :], (128, E)
        )
    )
    in_maps = []
    for c in range(N_CORES):
        xc = x[c * T_CORE : (c + 1) * T_CORE]  # [Tc, H] contiguous
        xh = xc.astype(np.float16)
        xl = xc - xh.astype(np.float32)
        xl8 = (xl * XL_S).astype(E4NP)
        # device layout [p, tile, k, t]: x[tt*128+t, k*128+p] -> A[p, tt, k, t]
        xh_dev = np.ascontiguousarray(
            xh.reshape(N_TILES, 128, KT, 128).transpose(3, 0, 2, 1)
        )
        xl8_dev = np.ascontiguousarray(
            xl8.reshape(N_TILES, 128, KT, 128).transpose(3, 0, 2, 1)
        )
        # pack per (p, tile): 14336B of xh then 7168B of xl8
        xz_dev = np.concatenate(
            [
                xh_dev.view(np.uint8).reshape(128, N_TILES, 14336),
                xl8_dev.view(np.uint8).reshape(128, N_TILES, 7168),
            ],
            axis=2,
        ).view(np.float16)
        in_maps.append(
            {
                "xz": xz_dev,
                "w16": w16_dev,
                "w8": w8_dev,
                "biasb": bias_b,
            }
        )
    return in_maps


def run(hidden_states, weight, e_score_correction_bias, trace=False, **spmd_kwargs):
    nc = _get_nc()
    in_maps = _prep_inputs(hidden_states, weight, e_score_correction_bias)
    res = run_bass_kernel_spmd(
        nc, in_maps, core_ids=list(range(N_CORES)), trace=trace, **spmd_kwargs
    )
    # outputs are [p, tile, 8] per core; token t = tile*128 + p
    idx = np.concatenate(
        [r["out_idx"].transpose(1, 0, 2).reshape(T_CORE, TOP_K) for r in res.results],
        axis=0,
    )
    w = np.concatenate(
        [r["out_w"].transpose(1, 0, 2).reshape(T_CORE, TOP_K) for r in res.results],
        axis=0,
    )
    return (idx.astype(np.int32), w.astype(np.float32)), res


def kernel(hidden_states, weight, e_score_correction_bias):
    (idx, w), _ = run(hidden_states, weight, e_score_correction_bias, trace=False)
    return idx, w


# revision 15
# speedup vs baseline: 1.0578x; 1.0475x over previous
"""DeepseekVL2 MoE gate (sigmoid + grouped top-k routing) on 8 trn2 cores.

Contract: kernel(**inputs) takes the FULL unsharded inputs
  hidden_states [4, 4096, 7168] f32, weight [256, 7168] f32,
  e_score_correction_bias [256] f32
and returns (topk_idx [16384, 8] int32, topk_weight [16384, 8] f32),
matching reference jax semantics.

Strategy:
  - Data parallel: 16384 tokens -> 2048 per core x 8 cores.
  - Gating GEMM as fp16 main pass + two fp8(e4m3) DoubleRow correction
    passes (2.0 fp16-units of PE work vs 3.0 for the fp16 hi/lo 3-pass):
      logits*1024 = xh@whs + (xl*128)@(whs/128) + (xh/8)@(wl*8)
    with xh=fp16(x), xl=x-xh, whs=fp16(w.T*1024), wl=w.T*1024-whs.
    Scales put every product in the same *1024 logit scale so all three
    passes accumulate into one PSUM bank. DoubleRow packs K=256 per
    fp8 matmul at 2x PE rate. Emulated: 3/16384 token mismatches.
    The (xh/8) fp8 operand is derived on-chip by a DVE cast pipelined
    ahead of its consumer; the (whs/128) plane ships from host (an
    on-chip cast of it was measurably less accurate).
  - DMA and tensor are nearly balanced (~50MB/core vs ~197us of PE
    work), so the schedule is built around the ramp: a chained warmup
    matmul burst opens the HAM clock gate (1.2 -> 2.4 GHz) before the
    first real matmul; tiles 0-2 run their fp16 pass first while only
    w16 + the fp16 half of their data has landed, with their fp8
    correction passes deferred (PSUM groups stay open) until the fp8
    weight planes arrive; x tiles ship as one packed plane (xh bytes
    then xl8 bytes -> single 21.5KB/partition run) split across the
    two HWDGE queues in measured-rate proportion.
  - Outputs accumulate in SBUF and leave as one DMA per tensor.
  - Routing per 128-token tile entirely on-chip with DVE max8 /
    max_index / match_replace ops (tie semantics match jax top_k).
"""

import numpy as np
import ml_dtypes

import concourse.bacc as bacc
import concourse.bass as bass
import concourse.mybir as mybir
from concourse.bass_utils import run_bass_kernel_spmd
from concourse.tile import TileContext

F16 = mybir.dt.float16
F32 = mybir.dt.float32
F8 = mybir.dt.float8e4
U32 = mybir.dt.uint32
I32 = mybir.dt.int32
E4NP = ml_dtypes.float8_e4m3

N_CORES = 8
T_FULL = 16384
T_CORE = T_FULL // N_CORES          # 2048
H = 7168
E = 256
KT = H // 128                        # 56 contraction tiles
NPAIR = KT // 2                      # 28 fp8 DoubleRow k-pairs per pass
N_TILES = T_CORE // 128              # 16 token tiles per core
N_WARMUP = 12                        # chained HAM warmup matmuls
N_GROUP = 8
GROUP_SIZE = E // N_GROUP            # 32
TOPK_GROUP = 4
TOP_K = 8
ROUTED_SCALING = 2.5
W_SCALE = 1024.0                     # keeps wl fp16-normal
XL_S = 128.0                         # xl plane stored as e4m3(xl*128)
X8_S = 0.125                         # x8 derived on-chip as e4m3(xh/8)
NEG_BIG = -1.0e30

# tensor-pass emission order: tiles 0-2 fp16-first (their fp8 passes are
# deferred until the fp8 weight planes have landed), stagger closed by t=8
_SCHED = (
    [("f16", 0), ("f16", 1), ("f16", 2), ("f8", 0), ("f16", 3), ("f8", 1),
     ("f16", 4), ("f8", 2), ("f16", 5), ("f8", 3), ("f8", 4), ("f16", 6),
     ("f8", 5), ("f16", 7), ("f8", 6), ("f16", 8), ("f8", 7), ("f8", 8)]
    + [p for t in range(9, N_TILES) for p in (("f16", t), ("f8", t))]
)


def _build_nc():
    nc = bacc.Bacc(
        "TRN2",
        target_bir_lowering=False,
        debug=False,
        num_devices=N_CORES,
    )

    # packed x plane, one run per (partition, tile): first 7168 f16 are
    # xh[k,t], the next 3584 f16 slots hold the 7168 xl8 fp8 bytes
    xz_d = nc.dram_tensor("xz", [128, N_TILES, 10752], F16, kind="ExternalInput").ap()
    # w16 pre-shuffled to [p, chunk, k, e]; w8 holds DoubleRow k-pairs
    # [p, chunk, pair, ko, e] with pairs 0..27 = wh8c, 28..55 = wl8.
    w16_d = nc.dram_tensor("w16", [128, 8, 7, E], F16, kind="ExternalInput").ap()
    w8_d = nc.dram_tensor("w8", [128, 4, 14, 2, E], F8, kind="ExternalInput").ap()
    bias_d = nc.dram_tensor("biasb", [128, E], F32, kind="ExternalInput").ap()
    # outputs in SBUF-native layout [p, tile, 8]; host re-folds to [T, 8]
    idx_d = nc.dram_tensor("out_idx", [128, N_TILES, TOP_K], U32, kind="ExternalOutput").ap()
    w_d = nc.dram_tensor("out_w", [128, N_TILES, TOP_K], F32, kind="ExternalOutput").ap()

    X = mybir.AxisListType.X
    Alu = mybir.AluOpType
    DR = mybir.MatmulPerfMode.DoubleRow

    with TileContext(nc) as tc:
        with (
            tc.tile_pool(name="wpool", bufs=1) as wpool,
            tc.tile_pool(name="xpool", bufs=3) as xpool,
            tc.tile_pool(name="x8pool", bufs=3) as x8pool,
            tc.tile_pool(name="spool", bufs=2) as spool,
            tc.tile_pool(name="small", bufs=2) as small,
            tc.tile_pool(name="psum", bufs=4, space="PSUM") as psum_pool,
        ):
            # HAM warmup: one chained accumulation of dependency-free matmuls
            # on zeroed scratch SBUF gives a contiguous ~5us busy window from
            # engine boot, so the clock gate opens before the first real
            # matmul. The PSUM result is never read.
            wsc = wpool.tile([128, 2 * E], F16, tag="wsc")
            nc.gpsimd.memset(wsc[:], 0.0)
            psw = psum_pool.tile([128, 2 * E], F32, tag="psw", name="psw")
            for i in range(N_WARMUP):
                nc.tensor.matmul(
                    psw[:], wsc[:, 0:128], wsc[:],
                    start=(i == 0), stop=(i == N_WARMUP - 1),
                    skip_group_check=True,
                )

            bias_sb = wpool.tile([128, E], F32, tag="bias")
            w16c = [
                wpool.tile([128, 7, E], F16, tag=f"w16c{c}", name=f"w16c{c}")
                for c in range(8)
            ]
            w8c = [
                wpool.tile([128, 14, 2, E], F8, tag=f"w8c{c}", name=f"w8c{c}")
                for c in range(4)
            ]
            # output accumulators, one slot per tile
            idxacc = wpool.tile([128, N_TILES, TOP_K], U32, tag="idxacc")
            wacc = wpool.tile([128, N_TILES, TOP_K], F32, tag="wacc")

            def w16_k(k):
                return w16c[k // 7][:, k % 7, :]

            def load_tile(tt, xz_t):
                if tt == 0:
                    # Ramp: w16/w8 chunks ride BOTH queues, interleaved with
                    # tile-0/1/2 pieces in first-use order so neither queue
                    # serializes the weight planes behind x data.
                    nc.sync.dma_start(xz_t[:, 0:1792], xz_d[:, 0, 0:1792])
                    nc.sync.dma_start(w16c[0][:], w16_d[:, 0, :, :])
                    nc.sync.dma_start(w16c[2][:], w16_d[:, 2, :, :])
                    nc.sync.dma_start(xz_t[:, 1792:3584], xz_d[:, 0, 1792:3584])
                    nc.sync.dma_start(w16c[4][:], w16_d[:, 4, :, :])
                    nc.sync.dma_start(w16c[6][:], w16_d[:, 6, :, :])
                    nc.scalar.dma_start(w16c[1][:], w16_d[:, 1, :, :])
                    nc.scalar.dma_start(w16c[3][:], w16_d[:, 3, :, :])
                    nc.scalar.dma_start(xz_t[:, 3584:7168], xz_d[:, 0, 3584:7168])
                    nc.scalar.dma_start(w16c[5][:], w16_d[:, 5, :, :])
                    nc.scalar.dma_start(w16c[7][:], w16_d[:, 7, :, :])
                    nc.scalar.dma_start(xz_t[:, 7168:10752], xz_d[:, 0, 7168:10752])
                    nc.scalar.dma_start(w8c[0][:], w8_d[:, 0, :, :, :])
                    nc.scalar.dma_start(w8c[1][:], w8_d[:, 1, :, :, :])
                elif tt == 1:
                    nc.sync.dma_start(xz_t[:, 0:7168], xz_d[:, 1, 0:7168])
                    nc.sync.dma_start(w8c[2][:], w8_d[:, 2, :, :, :])
                    nc.sync.dma_start(w8c[3][:], w8_d[:, 3, :, :, :])
                    nc.scalar.dma_start(xz_t[:, 7168:10752], xz_d[:, 1, 7168:10752])
                elif tt == 2:
                    nc.sync.dma_start(xz_t[:, 0:7168], xz_d[:, 2, 0:7168])
                    nc.scalar.dma_start(xz_t[:, 7168:10752], xz_d[:, 2, 7168:10752])
                    nc.scalar.dma_start(bias_sb[:], bias_d)
                else:
                    # whole-tile single DMA (21.5KB per-partition run),
                    # alternating queues; tile 3 goes to the lighter Q10
                    q = nc.scalar if tt % 2 == 1 else nc.sync
                    q.dma_start(xz_t[:], xz_d[:, tt, :])

            def xviews(xz_t):
                xh_v = xz_t[:, 0:7168].rearrange("p (k t) -> p k t", k=KT)
                xl8_v = (
                    xz_t[:, 7168:10752].bitcast(F8)
                    .rearrange("p (k t) -> p k t", k=KT)
                )
                return xh_v, xl8_v

            # prologue: loads + eager x8 casts for tiles 0..2
            xz_tiles = {}
            x8_tiles = {}
            ps_tiles = {}
            for t in range(3):
                xz_tiles[t] = xpool.tile([128, 10752], F16, tag="xz", name=f"xz{t}")
                load_tile(t, xz_tiles[t])
                x8_tiles[t] = x8pool.tile([128, KT, 128], F8, tag="x8", name=f"x8{t}")
                nc.vector.tensor_scalar(
                    x8_tiles[t][:], xviews(xz_tiles[t])[0], X8_S, None, op0=Alu.mult
                )

            def emit_f16(tt):
                if tt + 1 < N_TILES and tt + 1 not in xz_tiles:
                    xz_tiles[tt + 1] = xpool.tile(
                        [128, 10752], F16, tag="xz", name=f"xz{tt+1}"
                    )
                    load_tile(tt + 1, xz_tiles[tt + 1])
                xh_v, _ = xviews(xz_tiles[tt])
                ps = psum_pool.tile([128, E], F32, tag="ps", name=f"ps{tt}")
                ps_tiles[tt] = ps
                for k in range(KT):
                    nc.tensor.matmul(
                        ps[:], xh_v[:, k, :], w16_k(k),
                        start=(k == 0), stop=False, skip_group_check=True,
                    )

            def emit_f8_and_route(tt):
                _, xl8_v = xviews(xz_tiles[tt])
                x8_t = x8_tiles[tt]
                ps = ps_tiles[tt]
                for j in range(NPAIR):  # xl8 @ wh8c
                    nc.tensor.matmul(
                        ps[:], xl8_v[:, 2 * j : 2 * j + 2, :],
                        w8c[j // 14][:, j % 14, :, :],
                        start=False, stop=False, perf_mode=DR,
                        skip_group_check=True,
                    )
                for j in range(NPAIR):  # x8 @ wl8
                    nc.tensor.matmul(
                        ps[:], x8_t[:, 2 * j : 2 * j + 2, :],
                        w8c[2 + j // 14][:, j % 14, :, :],
                        start=False, stop=(j == NPAIR - 1), perf_mode=DR,
                        skip_group_check=True,
                    )

                # scores = sigmoid(logits) with the 1/1024 scale folded in
                scores = spool.tile([128, E], F32, tag="scores")
                nc.scalar.activation(
                    scores[:], ps[:],
                    mybir.ActivationFunctionType.Sigmoid,
                    scale=1.0 / W_SCALE,
                )

                # x8 cast for tile tt+2 goes on the DVE queue BEFORE this
                # tile's routing chain, so it is ready well before that
                # tile's fp8 w-correction matmuls (which can directly follow
                # this tile's in the closing-stagger schedule).
                tn = tt + 2
                if tn < N_TILES and tn not in xz_tiles:
                    xz_tiles[tn] = xpool.tile(
                        [128, 10752], F16, tag="xz", name=f"xz{tn}"
                    )
                    load_tile(tn, xz_tiles[tn])
                if tn < N_TILES and tn not in x8_tiles:
                    x8_tiles[tn] = x8pool.tile(
                        [128, KT, 128], F8, tag="x8", name=f"x8{tn}"
                    )
                    nc.vector.tensor_scalar(
                        x8_tiles[tn][:], xviews(xz_tiles[tn])[0], X8_S, None,
                        op0=Alu.mult,
                    )

                # scores_for_choice = scores + bias (bias varies along free dim)
                sfc = spool.tile([128, E], F32, tag="sfc")
                nc.vector.tensor_add(sfc[:], scores[:], bias_sb[:])

                # per-group top-2 sum: g1 = grouped max, remove it, g2 = max
                sfc_g = sfc[:].rearrange("p (g e) -> p g e", g=N_GROUP)
                g1 = small.tile([128, N_GROUP], F32, tag="g1")
                nc.vector.reduce_max(g1[:], sfc_g, axis=X)
                sfc_mr = spool.tile([128, E], F32, tag="scratch", name="sfc_mr")
                nc.vector.match_replace(sfc_mr[:], g1[:], sfc[:], NEG_BIG)
                g2 = small.tile([128, N_GROUP], F32, tag="g2")
                nc.vector.reduce_max(
                    g2[:], sfc_mr[:].rearrange("p (g e) -> p g e", g=N_GROUP), axis=X
                )
                gs = small.tile([128, N_GROUP], F32, tag="gs")
                nc.vector.tensor_add(gs[:], g1[:], g2[:])

                # top-4 groups: tau = 4th largest group score -> 0/1 mask
                gsrt = small.tile([128, 8], F32, tag="gsrt")
                nc.vector.max(out=gsrt[:], in_=gs[:])
                gmask = small.tile([128, N_GROUP], F32, tag="gmask")
                nc.vector.tensor_scalar(
                    gmask[:], gs[:], gsrt[:, TOPK_GROUP - 1 : TOPK_GROUP], None,
                    op0=Alu.is_ge,
                )

                # tmp = sfc * mask (expanded over the 32 experts of each group)
                tmp = spool.tile([128, E], F32, tag="tmp")
                nc.vector.tensor_mul(
                    tmp[:].rearrange("p (g e) -> p g e", g=N_GROUP),
                    sfc_g,
                    gmask[:].unsqueeze(2).to_broadcast([128, N_GROUP, GROUP_SIZE]),
                )

                # ordered top-8 of tmp (+ indices, jax tie order); indices land
                # directly in the output accumulator slot for this tile
                v8 = small.tile([128, 8], F32, tag="v8")
                nc.vector.max(out=v8[:], in_=tmp[:])
                nc.vector.max_index(idxacc[:, tt, :], v8[:], tmp[:])

                # mark the selected positions, pull raw sigmoid scores there
                tmp_mr = spool.tile([128, E], F32, tag="scratch", name="tmp_mr")
                nc.vector.match_replace(tmp_mr[:], v8[:], tmp[:], NEG_BIG)
                sel = spool.tile([128, E], F32, tag="sel")
                nc.vector.tensor_scalar(
                    sel[:], tmp_mr[:], NEG_BIG, None, op0=Alu.is_equal
                )
                scsel = spool.tile([128, E], F32, tag="scsel")
                nc.vector.tensor_mul(scsel[:], scores[:], sel[:])
                s8 = small.tile([128, 8], F32, tag="s8")
                nc.vector.max(out=s8[:], in_=scsel[:])
                s8i = small.tile([128, 8], U32, tag="s8i")
                nc.vector.max_index(s8i[:], s8[:], scsel[:])

                # re-pair scores to sfc order: w8[k] = sum_j s8[j]*(s8i[j]==i8[k])
                e8 = small.tile([128, 8, 8], F32, tag="e8")
                nc.vector.tensor_tensor(
                    e8[:],
                    s8i[:].unsqueeze(1).to_broadcast([128, 8, 8]),
                    idxacc[:, tt, :].unsqueeze(2).to_broadcast([128, 8, 8]),
                    op=Alu.is_equal,
                )
                w64 = small.tile([128, 8, 8], F32, tag="w64")
                nc.vector.tensor_mul(
                    w64[:], e8[:], s8[:].unsqueeze(1).to_broadcast([128, 8, 8])
                )
                w8v = small.tile([128, 8], F32, tag="w8v")
                nc.vector.reduce_sum(w8v[:], w64[:], axis=X)

                # normalize: w = w8v / sum * 2.5 (the reference's +1e-20 is
                # below fp32 ulp of the denominator, which is always >1)
                ds = small.tile([128, 1], F32, tag="ds")
                nc.vector.reduce_sum(ds[:], s8[:], axis=X)
                rcp = small.tile([128, 1], F32, tag="rcp")
                nc.vector.reciprocal(rcp[:], ds[:])
                nc.vector.tensor_scalar(
                    wacc[:, tt, :], w8v[:], rcp[:, 0:1], ROUTED_SCALING,
                    op0=Alu.mult, op1=Alu.mult,
                )

            for kind, tt in _SCHED:
                if kind == "f16":
                    emit_f16(tt)
                else:
                    emit_f8_and_route(tt)

            # single output DMA per tensor (keeps the HWDGE queues clean)
            nc.scalar.dma_start(idx_d[:], idxacc[:])
            nc.scalar.dma_start(w_d[:], wacc[:])

    nc.compile()
    return nc


_NC_CACHE = None


def _get_nc():
    global _NC_CACHE
    if _NC_CACHE is None:
        _NC_CACHE = _build_nc()
    return _NC_CACHE


def _prep_inputs(hidden_states, weight, e_score_correction_bias):
    x = np.ascontiguousarray(hidden_states, dtype=np.float32).reshape(T_FULL, H)
    wT = np.ascontiguousarray(weight, dtype=np.float32).T * W_SCALE  # [H, E]
    whs = wT.astype(np.float16)
    wl = wT - whs.astype(np.float32)
    w16_dev = np.ascontiguousarray(
        whs.reshape(8, 7, 128, E).transpose(2, 0, 1, 3)
    )
    # w8: DoubleRow pairs [p, chunk, pair, ko, e]; 28 wh8c pairs + 28 wl8
    wh8c = (whs.astype(np.float32) / XL_S).astype(E4NP)
    wl8 = (wl / X8_S).astype(E4NP)
    w8_all = np.concatenate(
        [wh8c.reshape(NPAIR, 2, 128, E), wl8.reshape(NPAIR, 2, 128, E)], axis=0
    )
    w8_dev = np.ascontiguousarray(
        w8_all.reshape(4, 14, 2, 128, E).transpose(3, 0, 1, 2, 4)
    )
    bias_b = np.ascontiguousarray(
        np.broadcast_to(
            np.asarray(e_score_correction_bias, dtype=np.float32)[None, :], (128, E)
        )
    )
    in_maps = []
    for c in range(N_CORES):
        xc = x[c * T_CORE : (c + 1) * T_CORE]  # [Tc, H] contiguous
        xh = xc.astype(np.float16)
        xl = xc - xh.astype(np.float32)
        xl8 = (xl * XL_S).astype(E4NP)
        # device layout [p, tile, k, t]: x[tt*128+t, k*128+p] -> A[p, tt, k, t]
        xh_dev = np.ascontiguousarray(
            xh.reshape(N_TILES, 128, KT, 128).transpose(3, 0, 2, 1)
        )
        xl8_dev = np.ascontiguousarray(
            xl8.reshape(N_TILES, 128, KT, 128).transpose(3, 0, 2, 1)
        )
        # pack per (p, tile): 14336B of xh then 7168B of xl8
        xz_dev = np.concatenate(
            [
                xh_dev.view(np.uint8).reshape(128, N_TILES, 14336),
                xl8_dev.view(np.uint8).reshape(128, N_TILES, 7168),
            ],
            axis=2,
        ).view(np.float16)
        in_maps.append(
            {
                "xz": xz_dev,
                "w16": w16_dev,
                "w8": w8_dev,
                "biasb": bias_b,
            }
        )
    return in_maps


def run(hidden_states, weight, e_score_correction_bias, trace=False, **spmd_kwargs):
    nc = _get_nc()
    in_maps = _prep_inputs(hidden_states, weight, e_score_correction_bias)
    res = run_bass_kernel_spmd(
        nc, in_maps, core_ids=list(range(N_CORES)), trace=trace, **spmd_kwargs
    )
    # outputs are [p, tile, 8] per core; token t = tile*128 + p
    idx = np.concatenate(
        [r["out_idx"].transpose(1, 0, 2).reshape(T_CORE, TOP_K) for r in res.results],
        axis=0,
    )
    w = np.concatenate(
        [r["out_w"].transpose(1, 0, 2).reshape(T_CORE, TOP_K) for r in res.results],
        axis=0,
    )
    return (idx.astype(np.int32), w.astype(np.float32)), res


def kernel(hidden_states, weight, e_score_correction_bias):
    (idx, w), _ = run(hidden_states, weight, e_score_correction_bias, trace=False)
    return idx, w
